# revision 15
# baseline (speedup 1.0000x reference)
"""DANetHead (position attention + channel attention + conv/BN/ReLU) on 8
Trainium2 NeuronCores via Bass/Tile.

Sharding: data-parallel over batch (4) x image-row-halves (2) = 8 cores.
Each core computes a 34-row window (32 own rows + 1 halo row on each side)
of one batch item.  The window is made position-uniform across cores by
rolling the pixel axis host-side, so a single SPMD program serves all 8
cores; per-core behaviour differs only through input data (rolled x, the
transposed residual, and a pad-row mask).

Per-core pipeline (all matmuls bf16 with fp32 PSUM accumulation):
  - q^T [64, 2176], k [64, 4096] projections.
  - v^T [4096, 513] (= x^T @ wv^T with a ones column appended).
  - energy computed TRANSPOSED: e^T[n,m] per 128-row n-chunk, exp on the
    Scalar engine straight from PSUM; the softmax denominator for each m
    falls out of the bmm against the ones column of v^T.
  - paT[m,c] accumulated over n-chunks, then normalized (per-partition
    reciprocal), residual-added and pad-masked in one pass; small PE
    transpose back to pa[c,m].
  - channel pooling partials + pair-wise AllReduce(add)/AllReduce(max),
    SE MLP + sigmoid on-chip (sigmoid via exp + reciprocal so only one
    activation table set is ever loaded).
  - 3x3 conv as 9 shifted-tap matmuls over a 66-column zero-padded layout,
    BN+ReLU fused into the final Scalar-engine activation.
"""

import numpy as np
import ml_dtypes

import concourse.bass as bass
import concourse.mybir as mybir
import concourse.tile as tile

BF16 = ml_dtypes.bfloat16
F32 = np.float32

P = 128
CIN = 512            # channels
NPIX = 4096          # 64*64 pixels
C8 = 64              # q/k channels
OC = 256             # conv output channels
M = 2176             # per-core pixel window: 34 rows * 64
NSUB = M // P        # 17
NCH = NPIX // P      # 32 n-chunks
BLOCKS = [(0, 512), (512, 512), (1024, 512), (1536, 512), (2048, 128)]
OWN_OFF = 64         # own pixels are window [64 : 64+2048] for every core
OWN = 2048
REPLICA_GROUPS = [[0, 1], [2, 3], [4, 5], [6, 7]]

BN_EPS = 1e-5

_BUILD_CACHE = {}


def _emit(tc, nc, t):
    """Emit the single-core program.  `t` maps input names -> dram handles."""
    fp32 = mybir.dt.float32
    bf16 = mybir.dt.bfloat16
    Act = mybir.ActivationFunctionType
    Alu = mybir.AluOpType
    X = mybir.AxisListType.X

    import contextlib
    ctx = contextlib.ExitStack()

    persist = ctx.enter_context(tc.tile_pool(name="persist", bufs=1))
    vt_pool = ctx.enter_context(tc.tile_pool(name="vt", bufs=NCH))
    xf_pool = ctx.enter_context(tc.tile_pool(name="xf", bufs=4))
    expt_pool = ctx.enter_context(tc.tile_pool(name="expt", bufs=3))
    patf_pool = ctx.enter_context(tc.tile_pool(name="patf", bufs=6))
    out_pool = ctx.enter_context(tc.tile_pool(name="yout", bufs=3))
    small = ctx.enter_context(tc.tile_pool(name="small", bufs=2))

    psum_e = ctx.enter_context(tc.tile_pool(name="ps_e", bufs=2, space="PSUM"))
    psum_pa = ctx.enter_context(tc.tile_pool(name="ps_pa", bufs=4, space="PSUM"))
    psum_d = ctx.enter_context(tc.tile_pool(name="ps_d", bufs=1, space="PSUM"))
    psum_tp = ctx.enter_context(tc.tile_pool(name="ps_tp", bufs=1, space="PSUM"))

    dram = ctx.enter_context(tc.tile_pool(name="dram", bufs=1, space="DRAM"))

    # ---------------- loads ----------------
    xf_sb = []
    for ci in range(4):
        xt = xf_pool.tile([P, NPIX], bf16, tag="xf")
        nc.sync.dma_start(out=xt, in_=t["xf"][ci * P:(ci + 1) * P, :])
        xf_sb.append(xt)

    xtr_sb = persist.tile([P, NSUB, CIN], fp32)
    nc.sync.dma_start(
        out=xtr_sb,
        in_=t["xtr"].ap().rearrange("(mi p) c -> p mi c", p=P))

    pmask_sb = persist.tile([P, NSUB], fp32)
    nc.sync.dma_start(
        out=pmask_sb, in_=t["pmask"].ap().rearrange("(mi p) -> p mi", p=P))

    wqT_sb = persist.tile([P, 4, C8], bf16)
    nc.sync.dma_start(out=wqT_sb,
                      in_=t["wqT"].ap().rearrange("(c p) h -> p c h", p=P))
    wkT_sb = persist.tile([P, 4, C8], bf16)
    nc.sync.dma_start(out=wkT_sb,
                      in_=t["wkT"].ap().rearrange("(c p) h -> p c h", p=P))
    wvT_sb = persist.tile([P, 4, CIN], bf16)
    nc.sync.dma_start(out=wvT_sb,
                      in_=t["wvT"].ap().rearrange("(c p) n -> p c n", p=P))

    bq_sb = persist.tile([C8, 1], fp32)
    nc.sync.dma_start(out=bq_sb, in_=t["bq"][:, :])
    bk_sb = persist.tile([C8, 1], fp32)
    nc.sync.dma_start(out=bk_sb, in_=t["bk"][:, :])

    w1T_sb = persist.tile([P, 4, C8], bf16)
    nc.sync.dma_start(out=w1T_sb,
                      in_=t["w1T"].ap().rearrange("(c p) h -> p c h", p=P))
    w2T_sb = persist.tile([C8, 4, P], bf16)
    nc.sync.dma_start(out=w2T_sb,
                      in_=t["w2T"].ap().rearrange("k (c p) -> k c p", p=P))

    cw_sb = persist.tile([P, 36, OC], bf16)
    nc.sync.dma_start(out=cw_sb,
                      in_=t["cw"].ap().rearrange("t (c p) o -> p (t c) o", p=P))

    bns_sb = persist.tile([P, 2], fp32)
    nc.sync.dma_start(out=bns_sb,
                      in_=t["bns"].ap().rearrange("(c p) one -> p (c one)", p=P))
    bnb_sb = persist.tile([P, 2], fp32)
    nc.sync.dma_start(out=bnb_sb,
                      in_=t["bnb"].ap().rearrange("(c p) one -> p (c one)", p=P))

    ident_sb = persist.tile([P, P], bf16)
    nc.sync.dma_start(out=ident_sb, in_=t["ident"][:, :])

    # ---------------- q / k projections ----------------
    qT_sb = persist.tile([C8, M], bf16)
    for off in range(0, M, 512):
        sz = min(512, M - off)
        q_ps = psum_e.tile([C8, 512], fp32, tag="e")
        for ci in range(4):
            nc.tensor.matmul(q_ps[:, :sz], lhsT=wqT_sb[:, ci, :],
                             rhs=xf_sb[ci][:, off:off + sz],
                             start=(ci == 0), stop=(ci == 3))
        nc.scalar.activation(qT_sb[:, off:off + sz], q_ps[:, :sz],
                             Act.Identity, bias=bq_sb[:, 0:1])

    k_sb = persist.tile([C8, NPIX], bf16)
    for off in range(0, NPIX, 512):
        k_ps = psum_e.tile([C8, 512], fp32, tag="e")
        for ci in range(4):
            nc.tensor.matmul(k_ps, lhsT=wkT_sb[:, ci, :],
                             rhs=xf_sb[ci][:, off:off + 512],
                             start=(ci == 0), stop=(ci == 3))
        nc.scalar.activation(k_sb[:, off:off + 512], k_ps,
                             Act.Identity, bias=bk_sb[:, 0:1])

    # ---------------- v^T (with ones column) ----------------
    vt_sb = []
    for nch in range(NCH):
        v_ps = psum_e.tile([P, 512], fp32, tag="e")
        for ci in range(4):
            nc.tensor.matmul(v_ps,
                             lhsT=xf_sb[ci][:, nch * P:(nch + 1) * P],
                             rhs=wvT_sb[:, ci, :],
                             start=(ci == 0), stop=(ci == 3))
        vt = vt_pool.tile([P, CIN + 1], bf16, tag="vt")
        nc.vector.tensor_copy(vt[:, 0:CIN], v_ps)
        nc.vector.memset(vt[:, CIN:CIN + 1], 1.0)
        vt_sb.append(vt)

    # ---------------- position attention ----------------
    pa_sb = persist.tile([P, 4, M], bf16)   # [c_part, c_chunk, m]

    # One persistent PSUM bank holds every softmax denominator; zero it once
    # with a PE matmul (so has_written is set on the whole bank), then each
    # (block, subchunk) accumulates into its own column with start=False.
    den_ps = psum_d.tile([P, 512], fp32, tag="den")
    zz = small.tile([P, 512], bf16, tag="zz", bufs=1)
    nc.vector.memset(zz, 0.0)
    nc.tensor.matmul(den_ps, lhsT=ident_sb, rhs=zz, start=True, stop=True)

    for bi, (boff, bsz) in enumerate(BLOCKS):
        nsub = bsz // P
        pa_ps = [psum_pa.tile([P, CIN], fp32, tag="pa_acc", name=f"pa_ps{j}")
                 for j in range(nsub)]

        for nch in range(NCH):
            e_ps = psum_e.tile([P, bsz], fp32, tag="e")
            nc.tensor.matmul(e_ps, lhsT=k_sb[:, nch * P:(nch + 1) * P],
                             rhs=qT_sb[:, boff:boff + bsz],
                             start=True, stop=True)
            expt = expt_pool.tile([P, bsz], bf16, tag="expt")
            nc.scalar.activation(expt, e_ps, Act.Exp)
            for j in range(nsub):
                lhs = expt[:, j * P:(j + 1) * P]
                nc.tensor.matmul(pa_ps[j][:, 0:CIN], lhsT=lhs,
                                 rhs=vt_sb[nch][:, 0:CIN],
                                 start=(nch == 0), stop=(nch == NCH - 1))
                col = bi * 4 + j
                nc.tensor.matmul(den_ps[:, col:col + 1], lhsT=lhs,
                                 rhs=vt_sb[nch][:, CIN:CIN + 1],
                                 start=False, stop=False,
                                 skip_group_check=True)

        # normalize + pad-mask + residual, then transpose back to [c, m]
        recip = small.tile([P, 4], fp32, tag="recip")
        nc.vector.reciprocal(recip[:, 0:nsub],
                             den_ps[:, bi * 4:bi * 4 + nsub])
        jg0 = boff // P
        nc.vector.tensor_mul(recip[:, 0:nsub], recip[:, 0:nsub],
                             pmask_sb[:, jg0:jg0 + nsub])
        for j in range(nsub):
            jg = jg0 + j
            # ACT: paT_unnorm * recip (per-partition scale pointer), then
            # DVE adds the residual — keeps every pointer-op at <=1 wait.
            pnorm = patf_pool.tile([P, CIN], fp32, tag="pnorm")
            nc.scalar.activation(pnorm, pa_ps[j], Act.Copy,
                                 scale=recip[:, j:j + 1])
            patf = patf_pool.tile([P, CIN], bf16, tag="patf")
            nc.vector.tensor_add(patf, pnorm, xtr_sb[:, jg, :])
            for cc in range(4):
                tp_ps = psum_tp.tile([P, P], bf16, tag="tp")
                nc.tensor.transpose(tp_ps, patf[:, cc * P:(cc + 1) * P],
                                    ident_sb)
                nc.vector.tensor_copy(pa_sb[:, cc, jg * P:(jg + 1) * P], tp_ps)

    # ---------------- channel pooling + pair AllReduce ----------------
    sums_sb = small.tile([P, 4], fp32, tag="pool_s")
    maxs_sb = small.tile([P, 4], fp32, tag="pool_m")
    for cc in range(4):
        nc.vector.reduce_sum(sums_sb[:, cc:cc + 1],
                             pa_sb[:, cc, OWN_OFF:OWN_OFF + OWN], axis=X)
        nc.vector.reduce_max(maxs_sb[:, cc:cc + 1],
                             pa_sb[:, cc, OWN_OFF:OWN_OFF + OWN], axis=X)

    sums_d = dram.tile([CIN], fp32, tag="sums_d")
    maxs_d = dram.tile([CIN], fp32, tag="maxs_d")
    sums_r = dram.tile([CIN], fp32, tag="sums_r")
    maxs_r = dram.tile([CIN], fp32, tag="maxs_r")
    nc.gpsimd.dma_start(out=sums_d.rearrange("(c p) -> p c", p=P), in_=sums_sb)
    nc.gpsimd.dma_start(out=maxs_d.rearrange("(c p) -> p c", p=P), in_=maxs_sb)
    nc.gpsimd.collective_compute("AllReduce", Alu.add,
                                 replica_groups=REPLICA_GROUPS,
                                 ins=[sums_d.opt()], outs=[sums_r.opt()])
    nc.gpsimd.collective_compute("AllReduce", Alu.max,
                                 replica_groups=REPLICA_GROUPS,
                                 ins=[maxs_d.opt()], outs=[maxs_r.opt()])
    zs_sb = small.tile([P, 4], fp32, tag="zs")
    zm_sb = small.tile([P, 4], fp32, tag="zm")
    nc.gpsimd.dma_start(out=zs_sb, in_=sums_r.rearrange("(c p) -> p c", p=P))
    nc.gpsimd.dma_start(out=zm_sb, in_=maxs_r.rearrange("(c p) -> p c", p=P))

    # ---------------- SE MLP + sigmoid ----------------
    rhs_z = small.tile([P, 4, 2], bf16, tag="rhs_z")
    nc.scalar.mul(rhs_z[:, :, 0], zs_sb, 1.0 / float(NPIX))
    nc.vector.tensor_copy(rhs_z[:, :, 1], zm_sb)

    h_ps = psum_d.tile([C8, 2], fp32, tag="den")
    for cc in range(4):
        nc.tensor.matmul(h_ps, lhsT=w1T_sb[:, cc, :], rhs=rhs_z[:, cc, :],
                         start=(cc == 0), stop=(cc == 3))
    h_sb = small.tile([C8, 2], bf16, tag="h_sb")
    nc.scalar.activation(h_sb, h_ps, Act.Relu)

    stot = small.tile([P, 4], fp32, tag="stot")
    s_sb = small.tile([P, 4, 2], fp32, tag="s_sb", bufs=1)
    for cc in range(4):
        s_ps = psum_pa.tile([P, 2], fp32, tag="pa_acc")
        nc.tensor.matmul(s_ps, lhsT=w2T_sb[:, cc, :], rhs=h_sb,
                         start=True, stop=True)
        nc.vector.tensor_copy(s_sb[:, cc, :], s_ps)
        nc.vector.tensor_add(stot[:, cc:cc + 1], s_sb[:, cc, 0:1],
                             s_sb[:, cc, 1:2])

    es = small.tile([P, 4], fp32, tag="es")
    nc.scalar.activation(es, stot, Act.Exp, scale=-1.0)
    nc.vector.tensor_scalar_add(es, es, 1.0)
    scale_sb = small.tile([P, 4], fp32, tag="scale")
    nc.vector.reciprocal(scale_sb, es)

    # ---------------- ca buffer (34 rows x 66 cols, zero col pads) ----------
    ca_sb = persist.tile([P, 4, 34 * 66], bf16)
    for cc in range(4):
        cav = ca_sb[:, cc, :].rearrange("p (r x) -> p r x", x=66)
        nc.vector.memset(cav[:, :, 0:1], 0.0)
        nc.vector.memset(cav[:, :, 65:66], 0.0)
        nc.vector.tensor_scalar(
            out=cav[:, :, 1:65],
            in0=pa_sb[:, cc, :].rearrange("p (r x) -> p r x", x=64),
            scalar1=scale_sb[:, cc:cc + 1], scalar2=None, op0=Alu.mult)

    # ---------------- conv 3x3 + BN + ReLU ----------------
    for pt in range(4):
        for oc in range(2):
            y_ps = psum_pa.tile([P, 512], fp32, tag="pa_acc")
            idx = 0
            for kh in range(3):
                for kw in range(3):
                    tnum = 3 * kh + kw
                    rs = 1 + 8 * pt + (kh - 1)
                    for ci in range(4):
                        rhs = (ca_sb[:, ci, :]
                               .rearrange("p (r x) -> p r x", x=66)
                               [:, rs:rs + 8, kw:kw + 64])
                        nc.tensor.matmul(
                            y_ps, lhsT=cw_sb[:, tnum * 4 + ci,
                                             oc * P:(oc + 1) * P],
                            rhs=rhs, start=(idx == 0), stop=(idx == 35))
                        idx += 1
            y_sb = out_pool.tile([P, 512], fp32, tag="y_sb")
            nc.scalar.activation(y_sb, y_ps, Act.Relu,
                                 bias=bnb_sb[:, oc:oc + 1],
                                 scale=bns_sb[:, oc:oc + 1])
            nc.sync.dma_start(
                out=t["out"][oc * P:(oc + 1) * P, pt * 512:(pt + 1) * 512],
                in_=y_sb)

    ctx.close()


def build():
    """Build (and cache) the SPMD Bass program."""
    if "nc" in _BUILD_CACHE:
        return _BUILD_CACHE["nc"]
    from concourse import bacc
    nc = bacc.Bacc("TRN2", target_bir_lowering=False, num_devices=8)
    f32 = mybir.dt.float32
    bf16 = mybir.dt.bfloat16
    t = {}
    t["xf"] = nc.dram_tensor("xf", [CIN, NPIX], bf16, kind="ExternalInput")
    t["xtr"] = nc.dram_tensor("xtr", [M, CIN], f32, kind="ExternalInput")
    t["pmask"] = nc.dram_tensor("pmask", [M], f32, kind="ExternalInput")
    t["wqT"] = nc.dram_tensor("wqT", [CIN, C8], bf16, kind="ExternalInput")
    t["wkT"] = nc.dram_tensor("wkT", [CIN, C8], bf16, kind="ExternalInput")
    t["wvT"] = nc.dram_tensor("wvT", [CIN, CIN], bf16, kind="ExternalInput")
    t["bq"] = nc.dram_tensor("bq", [C8, 1], f32, kind="ExternalInput")
    t["bk"] = nc.dram_tensor("bk", [C8, 1], f32, kind="ExternalInput")
    t["w1T"] = nc.dram_tensor("w1T", [CIN, C8], bf16, kind="ExternalInput")
    t["w2T"] = nc.dram_tensor("w2T", [C8, CIN], bf16, kind="ExternalInput")
    t["cw"] = nc.dram_tensor("cw", [9, CIN, OC], bf16, kind="ExternalInput")
    t["bns"] = nc.dram_tensor("bns", [OC, 1], f32, kind="ExternalInput")
    t["bnb"] = nc.dram_tensor("bnb", [OC, 1], f32, kind="ExternalInput")
    t["ident"] = nc.dram_tensor("ident", [P, P], bf16, kind="ExternalInput")
    t["out"] = nc.dram_tensor("out", [OC, OWN], f32, kind="ExternalOutput")

    with tile.TileContext(nc) as tc:
        _emit(tc, nc, t)
    nc.compile()

    _BUILD_CACHE["nc"] = nc
    return nc


def make_in_maps(x, wq, bq, wk, bk, wv, bv, ca_w1, ca_w2, conv_w,
                 bn_gamma, bn_beta, bn_mean, bn_var):
    x = np.ascontiguousarray(np.asarray(x, F32))
    B = x.shape[0]
    xf_full = x.reshape(B, CIN, NPIX)

    common = {
        "wqT": np.ascontiguousarray(np.asarray(wq, F32).T.astype(BF16)),
        "wkT": np.ascontiguousarray(np.asarray(wk, F32).T.astype(BF16)),
        "wvT": np.ascontiguousarray(np.asarray(wv, F32).T.astype(BF16)),
        "bq": np.asarray(bq, F32).reshape(C8, 1),
        "bk": np.asarray(bk, F32).reshape(C8, 1),
        "w1T": np.ascontiguousarray(np.asarray(ca_w1, F32).T.astype(BF16)),
        "w2T": np.ascontiguousarray(np.asarray(ca_w2, F32).T.astype(BF16)),
        "cw": np.ascontiguousarray(np.stack(
            [np.asarray(conv_w, F32)[:, :, kh, kw].T
             for kh in range(3) for kw in range(3)]).astype(BF16)),
        "ident": np.eye(P, dtype=BF16),
    }
    bns = (np.asarray(bn_gamma, F32)
           / np.sqrt(np.asarray(bn_var, F32) + BN_EPS)).astype(F32)
    bnb = (np.asarray(bn_beta, F32) - np.asarray(bn_mean, F32) * bns).astype(F32)
    common["bns"] = bns.reshape(OC, 1)
    common["bnb"] = bnb.reshape(OC, 1)

    bv_f = np.asarray(bv, F32)
    in_maps = []
    for core in range(8):
        b, h = core // 2, core % 2
        r0 = 32 * h - 1                       # first window row (may be -1)
        rolled = np.roll(xf_full[b], -r0 * 64, axis=1)
        xtr = rolled[:, :M].T + bv_f[None, :]
        pmask = np.ones((M,), F32)
        if h == 0:
            xtr[0:64] = 0.0
            pmask[0:64] = 0.0
        else:
            xtr[M - 64:M] = 0.0
            pmask[M - 64:M] = 0.0
        in_maps.append(dict(
            common,
            xf=np.ascontiguousarray(rolled.astype(BF16)),
            xtr=np.ascontiguousarray(xtr.astype(F32)),
            pmask=pmask,
        ))
    return in_maps


def assemble(results):
    out = np.zeros((4, OC, 64, 64), F32)
    for core in range(8):
        b, h = core // 2, core % 2
        out[b, :, 32 * h:32 * h + 32, :] = \
            results[core]["out"].reshape(OC, 32, 64)
    return out


def kernel(**inputs):
    from concourse.bass_utils import run_bass_kernel_spmd
    nc = build()
    in_maps = make_in_maps(**inputs)
    res = run_bass_kernel_spmd(nc, in_maps, core_ids=list(range(8)))
    return assemble(res.results)
